# revision 9
# baseline (speedup 1.0000x reference)
# kernel.py — MABSINK (Sinkhorn attention block) Trainium2 Bass kernel.
# Self-contained: hardcodes shapes B=8, n=1024, dQ=dV=512, H=8; shards batch
# across 8 NeuronCores (1 batch element per core), runs SPMD, gathers output.
#
# Math (per core, per head h; Q_h = (Q @ Wq.T + bq)[:, h*64:(h+1)*64]):
#   S   = Q_h Q_h^T / sqrt(512)            (symmetric!)
#   E   = exp(S);  r_i = sum_j E_ij;  c_j = sum_i E_ij / r_i
#   A   = n*mu' * E_ij / (r_i c_j),  mu' = 1/n + 1e-8
#   O_h = Q_h + A @ Q_h
# then head-recombine -> LN0 -> x + relu(x@Wo.T+bo) -> LN1.
#
# V2 layout/engine plan:
#  - everything stays transposed ([feature-part, token-free]) until the very
#    end; both LayerNorms run in transposed layout with stats via ones-matmul.
#  - exp runs as one 1024-wide ACT op per row-chunk with accum_out giving r
#    directly; Wt = E*invr is computed in place on E (DVE, bf16 2x mode) with
#    accum_out giving c.
#  - copies are split between ACT/DVE to balance engine load; bf16 used on
#    every DVE op that allows the 2x packed mode.

import math
import numpy as np

B, N, DQ, DV, H = 8, 1024, 512, 512, 8
D = DV // H          # 64 head dim
P = 128
NRC = N // P         # 8 row chunks
NCC = DV // P        # 4 feature chunks
LN_EPS = 1e-5
SCALE_S = 1.0 / math.sqrt(DV)
AFACT = N * (1.0 / N + 1e-8)   # n * mu'

_CACHE = {}


def _build(mm_bf16=True, reps=1):
    import concourse.mybir as mybir
    from concourse import bacc
    import concourse.tile as tile
    from concourse.masks import make_identity
    from contextlib import ExitStack

    f32 = mybir.dt.float32
    bf = mybir.dt.bfloat16
    AF = mybir.ActivationFunctionType
    OP = mybir.AluOpType

    nc = bacc.Bacc()
    dQ = nc.dram_tensor("Q", [N, DQ], f32, kind="ExternalInput")
    dWq = nc.dram_tensor("Wq", [DV, DQ], f32, kind="ExternalInput")
    dbq = nc.dram_tensor("bq", [DQ], f32, kind="ExternalInput")
    dWo = nc.dram_tensor("Wo", [DV, DV], f32, kind="ExternalInput")
    dbo = nc.dram_tensor("bo", [DV], f32, kind="ExternalInput")
    dg0 = nc.dram_tensor("g0", [DV], f32, kind="ExternalInput")
    db0 = nc.dram_tensor("b0", [DV], f32, kind="ExternalInput")
    dg1 = nc.dram_tensor("g1", [DV], f32, kind="ExternalInput")
    db1 = nc.dram_tensor("b1", [DV], f32, kind="ExternalInput")
    dout = nc.dram_tensor("out", [N, DV], f32, kind="ExternalOutput")

    with tile.TileContext(nc) as tc, ExitStack() as ctx:
        pc = ctx.enter_context(tc.tile_pool(name="pc", bufs=1))
        pin = ctx.enter_context(tc.tile_pool(name="pin", bufs=2))
        pbig = ctx.enter_context(tc.tile_pool(name="pbig", bufs=1))
        pqptb = ctx.enter_context(tc.tile_pool(name="pqptb", bufs=4))
        pE = ctx.enter_context(tc.tile_pool(name="pE", bufs=16))
        prep = ctx.enter_context(tc.tile_pool(name="prep", bufs=2))
        psm = ctx.enter_context(tc.tile_pool(name="psm", bufs=3))
        pot = ctx.enter_context(tc.tile_pool(name="pot", bufs=4))
        pstat = ctx.enter_context(tc.tile_pool(name="pstat", bufs=2))
        pffn = ctx.enter_context(tc.tile_pool(name="pffn", bufs=4))
        pout = ctx.enter_context(tc.tile_pool(name="pout", bufs=2))

        # PSUM: 8 banks total.
        pp_s = ctx.enter_context(tc.tile_pool(name="pp_s", bufs=2, space="PSUM"))
        pp_a = ctx.enter_context(tc.tile_pool(name="pp_a", bufs=2, space="PSUM"))
        pp_r = ctx.enter_context(tc.tile_pool(name="pp_r", bufs=1, space="PSUM"))

        def psum_s(name):     # [128,1024] f32, 2 banks, 2 bufs
            return pp_s.tile([P, N], f32, tag="s", name=name)

        def psum_a(name):     # [128,512] f32, 1 bank, 2 bufs
            return pp_a.tile([P, DV], f32, tag="a", name=name)

        def psum_aq(name):    # [128,1024] f32, 2 banks, 1 buf
            return pp_r.tile([P, N], f32, tag="aq", name=name)

        # ---- constants -------------------------------------------------
        ident_f = pc.tile([P, P], f32, tag="ident_f")
        make_identity(nc, ident_f)
        ident_b = pc.tile([P, P], bf, tag="ident_b")
        make_identity(nc, ident_b)
        ones_bf = pc.tile([P, P], bf, tag="ones_bf")
        nc.vector.memset(ones_bf, 1.0)
        zero_col = pc.tile([P, 1], f32, tag="zero_col")
        nc.vector.memset(zero_col, 0.0)
        eps_col = pc.tile([P, 1], f32, tag="eps_col")
        nc.vector.memset(eps_col, LN_EPS)
        nc.const_aps.aps[(f32, 0.0)] = zero_col
        nc.const_aps.aps[(f32, LN_EPS)] = eps_col
        # SEL[p, c*128+m] = (p == c): replicates row c of an [8,128] rhs
        # across all 128 output partitions via matmul.
        sel = pc.tile([NRC, NRC * P], bf, tag="sel")
        nc.gpsimd.memset(sel, 0.0)
        nc.gpsimd.affine_select(
            out=sel.rearrange("p (c m) -> p c m", m=P),
            in_=sel.rearrange("p (c m) -> p c m", m=P),
            compare_op=mybir.AluOpType.not_equal,
            fill=1.0, base=0,
            pattern=[[-1, NRC], [0, P]],
            channel_multiplier=1,
        )

        # per-partition column layouts [128, 4] (col cc = feature chunk cc)
        def col_vec(dvec, tag):
            v4 = pc.tile([NCC, P], f32, tag=tag + "4")
            nc.sync.dma_start(v4, dvec.rearrange("(c p) -> c p", p=P))
            pst = psum_a("colv_ps")
            nc.tensor.transpose(pst[:, :NCC], v4, ident_f[:NCC, :NCC])
            col = pc.tile([P, NCC], f32, tag=tag + "c")
            nc.scalar.activation(col, pst[:, :NCC], AF.Copy)
            return col

        bq_col = col_vec(dbq, "bq")
        g0_col = col_vec(dg0, "g0")
        b0_col = col_vec(db0, "b0")
        bo_col = col_vec(dbo, "bo")
        g1_col = col_vec(dg1, "g1")
        b1_col = col_vec(db1, "b1")

        for _rep in range(reps):
            # ---- load + transpose inputs -------------------------------
            # QTb[:, kc*1024 + i] = Q[i, kc*128 + p]  (k on partitions)
            QTb = pbig.tile([P, NCC * N], bf, tag="qtb", name="QTb")
            QTv = QTb.rearrange("p (k i) -> p k i", k=NCC)
            for rc in range(NRC):
                qsb = pin.tile([P, DQ], f32, tag="qsb")
                nc.sync.dma_start(qsb, dQ[rc * P:(rc + 1) * P, :])
                qsb_b = pin.tile([P, DQ], bf, tag="qsb_b")
                nc.vector.tensor_copy(qsb_b, qsb)
                psT = pp_a.tile([P, DQ], bf, tag="a", name="qT_ps")
                for kc in range(NCC):
                    nc.tensor.transpose(psT[:, kc * P:(kc + 1) * P],
                                        qsb_b[:, kc * P:(kc + 1) * P], ident_b)
                nc.vector.tensor_copy(QTv[:, :, rc * P:(rc + 1) * P],
                                      psT.rearrange("p (k m) -> p k m", k=NCC))

            # WqTb[:, kc*512 + d] = Wq[d, kc*128 + p] ; same for Wo
            WqTb = pbig.tile([P, NCC * DV], bf, tag="wqtb", name="WqTb")
            WoTb = pbig.tile([P, NCC * DV], bf, tag="wotb", name="WoTb")
            for src, dst in ((dWq, WqTb), (dWo, WoTb)):
                dstv = dst.rearrange("p (k d) -> p k d", k=NCC)
                for dc in range(NCC):
                    wsb = pin.tile([P, DQ], f32, tag="qsb")
                    nc.sync.dma_start(wsb, src[dc * P:(dc + 1) * P, :])
                    wsb_b = pin.tile([P, DQ], bf, tag="qsb_b")
                    nc.vector.tensor_copy(wsb_b, wsb)
                    psT = pp_a.tile([P, DQ], bf, tag="a", name="wT_ps")
                    for kc in range(NCC):
                        nc.tensor.transpose(psT[:, kc * P:(kc + 1) * P],
                                            wsb_b[:, kc * P:(kc + 1) * P], ident_b)
                    nc.vector.tensor_copy(dstv[:, :, dc * P:(dc + 1) * P],
                                          psT.rearrange("p (k m) -> p k m", k=NCC))

            # ---- QpT (transposed Qp, bf16) then Qp (row-major) ----------
            # QpTb[cc][p, i] = Qp[i, cc*128+p],  Qp = Q @ Wq.T + bq
            QpTb = [pqptb.tile([P, N], bf, tag="qptb", name="qptb") for _ in range(NCC)]
            for cc in range(NCC):
                psQT = psum_s("qpt_ps")
                for hf in range(2):
                    for kc in range(NCC):
                        nc.tensor.matmul(
                            psQT[:, hf * DV:(hf + 1) * DV],
                            WqTb[:, kc * DV + cc * P: kc * DV + (cc + 1) * P],
                            QTv[:, kc, hf * DV:(hf + 1) * DV],
                            start=(kc == 0), stop=(kc == NCC - 1))
                nc.scalar.activation(QpTb[cc], psQT, AF.Identity,
                                     bias=bq_col[:, cc:cc + 1])

            # Qp_big[:, jc*512 + d] = Qp[jc*128 + p, d] * AFACT  (bf16)
            Qp = pbig.tile([P, NRC * DV], bf, tag="qp", name="Qp")
            Qpv = Qp.rearrange("p (j d) -> p j d", j=NRC)
            for rc in range(NRC):
                psP = pp_a.tile([P, DV], bf, tag="a", name="qp_ps")
                for cc in range(NCC):
                    nc.tensor.transpose(psP[:, cc * P:(cc + 1) * P],
                                        QpTb[cc][:, rc * P:(rc + 1) * P], ident_b)
                nc.vector.tensor_scalar_mul(Qpv[:, rc, :], psP, AFACT)

            # ---- per-head Sinkhorn attention ----------------------------
            OT = [pot.tile([P, N], bf, tag="ot", name="ot") for _ in range(NCC)]
            for h in range(H):
                tb = h // 2
                po = (h % 2) * D
                qht = QpTb[tb][po:po + D, :]

                # E = exp(S/sqrt(dv)); r = rowsum (accum)
                E = [pE.tile([P, N], bf, tag="E", name="E") for _ in range(NRC)]
                r_mat = psm.tile([P, NRC], f32, tag="r_mat")
                for ci in range(NRC):
                    psS = psum_s("s_ps")
                    for hf in range(2):
                        nc.tensor.matmul(psS[:, hf * DV:(hf + 1) * DV],
                                         qht[:, ci * P:(ci + 1) * P],
                                         qht[:, hf * DV:(hf + 1) * DV],
                                         start=True, stop=True)
                    nc.scalar.activation(E[ci], psS, AF.Exp, scale=SCALE_S,
                                         accum_out=r_mat[:, ci:ci + 1])

                # invr in chunk-column layout (for the c matvec) ...
                invr = psm.tile([P, NRC], f32, tag="invr")
                nc.vector.reciprocal(invr, r_mat)
                invr_bf = psm.tile([P, NRC], bf, tag="invr_bf")
                nc.vector.tensor_copy(invr_bf, invr)
                # ... and replicated along the free axis (for scaling A@Q)
                pst = psum_a("invr_t_ps")
                nc.tensor.transpose(pst[:NRC, :P], invr, ident_f)
                sbt = psm.tile([NRC, P], bf, tag="sbt")
                nc.scalar.activation(sbt, pst[:NRC, :P], AF.Copy)
                rep = [psum_a("rep_ps") for _ in range(2)]
                for c in range(NRC):
                    nc.tensor.matmul(rep[c // 4][:, (c % 4) * P:(c % 4 + 1) * P],
                                     sel[:, c * P:(c + 1) * P], sbt,
                                     start=True, stop=True)
                invr_rep = prep.tile([P, N], bf, tag="invr_rep")
                nc.scalar.activation(invr_rep[:, :DV], rep[0], AF.Copy)
                nc.vector.tensor_copy(invr_rep[:, DV:], rep[1])

                # c_j = sum_i E_ij * invr_i  via PE matvec (symmetry: E = E^T)
                psC = [psum_a("c_ps") for _ in range(2)]
                for hf in range(2):
                    for ci in range(NRC):
                        nc.tensor.matmul(psC[hf][:1, :],
                                         invr_bf[:, ci:ci + 1],
                                         E[ci][:, hf * DV:(hf + 1) * DV],
                                         start=(ci == 0), stop=(ci == NRC - 1))
                c_row = psm.tile([1, N], f32, tag="c_row")
                nc.vector.tensor_copy(c_row[:, :DV], psC[0][:1, :])
                nc.vector.tensor_copy(c_row[:, DV:], psC[1][:1, :])
                # transpose c back to chunk-column layout, then reciprocal
                psT = psum_a("c_t_ps")
                for ci in range(NRC):
                    nc.tensor.transpose(psT[:, ci:ci + 1],
                                        c_row[:, ci * P:(ci + 1) * P],
                                        ident_f[:1, :1])
                c_mat = psm.tile([P, NRC], f32, tag="c_mat")
                nc.scalar.activation(c_mat, psT[:, :NRC], AF.Copy)
                invc = psm.tile([P, NRC], f32, tag="invc")
                nc.vector.reciprocal(invc, c_mat)

                # Qc = Qp_head * invc  (AFACT already folded into Qp)
                Qc = psm.tile([P, DV], bf, tag="qc")
                for jc in range(NRC):
                    nc.vector.tensor_scalar_mul(
                        Qc[:, jc * D:(jc + 1) * D], Qpv[:, jc, h * D:(h + 1) * D],
                        invc[:, jc:jc + 1])

                # (A@Q)^T pre-invr = sum_j Qc[j,:]^T E[j,:]  -> [64, 1024]
                psA = psum_aq("aq_ps")
                for jc in range(NRC):
                    for hf in range(2):
                        nc.tensor.matmul(psA[:D, hf * DV:(hf + 1) * DV],
                                         Qc[:, jc * D:(jc + 1) * D],
                                         E[jc][:, hf * DV:(hf + 1) * DV],
                                         start=(jc == 0), stop=(jc == NRC - 1))
                # O^T = invr * (A@Q)^T + Q^T  (invr rides the free axis)
                t64 = psm.tile([P, N], bf, tag="t64")
                nc.vector.tensor_tensor(t64[po:po + D, :], psA[:D, :],
                                        invr_rep[po:po + D, :], OP.mult)
                nc.vector.tensor_tensor(OT[tb][po:po + D, :], t64[po:po + D, :],
                                        qht, OP.add)

            # ---- transposed layer norm helper ---------------------------
            def t_layernorm(SRC, DSTpool, dst_tag, g_col, b_col):
                """LN over the feature axis (= partitions). SRC: 4 bf16
                [128,1024] tiles. Returns 4 bf16 [128,1024] tiles."""
                psL = psum_s("lnm_ps")
                psQ = psum_s("lnq_ps")
                for hf in range(2):
                    sl = slice(hf * DV, (hf + 1) * DV)
                    for cc in range(NCC):
                        nc.tensor.matmul(psL[:, sl], ones_bf, SRC[cc][:, sl],
                                         start=(cc == 0), stop=(cc == NCC - 1))
                for hf in range(2):
                    sl = slice(hf * DV, (hf + 1) * DV)
                    for cc in range(NCC):
                        sqh = pstat.tile([P, DV], bf, tag="sqh", name="sqh")
                        nc.vector.tensor_tensor(sqh, SRC[cc][:, sl],
                                                SRC[cc][:, sl], OP.mult)
                        nc.tensor.matmul(psQ[:, sl], ones_bf, sqh,
                                         start=(cc == 0), stop=(cc == NCC - 1))
                m_rep = pstat.tile([P, N], bf, tag="m_rep")
                nc.scalar.activation(m_rep, psL, AF.Copy, scale=1.0 / DV)
                q_rep = pstat.tile([P, N], bf, tag="q_rep")
                nc.scalar.activation(q_rep, psQ, AF.Copy, scale=1.0 / DV)
                msq = pstat.tile([P, N], bf, tag="msq")
                nc.vector.tensor_tensor(msq, m_rep, m_rep, OP.mult)
                var = pstat.tile([P, N], bf, tag="var")
                nc.vector.tensor_tensor(var, q_rep, msq, OP.subtract)
                sd = pstat.tile([P, N], bf, tag="sd")
                nc.scalar.activation(sd, var, AF.Sqrt, bias=LN_EPS)
                rstd = pstat.tile([P, N], bf, tag="rstd")
                with nc.allow_low_precision(reason="bf16 rstd within rel-err budget"):
                    nc.vector.reciprocal(rstd, sd)
                nsr = pstat.tile([P, N], bf, tag="nsr")
                nc.vector.tensor_tensor(nsr, m_rep, rstd, OP.mult)
                out = []
                for cc in range(NCC):
                    x1 = pstat.tile([P, N], bf, tag="ln_t", name="ln_t")
                    nc.vector.tensor_tensor(x1, SRC[cc], rstd, OP.mult)
                    x2 = pstat.tile([P, N], bf, tag="ln_u", name="ln_u")
                    nc.vector.tensor_tensor(x2, x1, nsr, OP.subtract)
                    o = DSTpool.tile([P, N], bf, tag=dst_tag, name=dst_tag)
                    nc.vector.tensor_scalar(o, x2, g_col[:, cc:cc + 1],
                                            b_col[:, cc:cc + 1], OP.mult, OP.add)
                    out.append(o)
                return out

            # ---- LN0 -----------------------------------------------------
            O1 = t_layernorm(OT, pot, "o1t", g0_col, b0_col)

            # ---- FFN: O2T = O1T + relu(Wo @ O1T + bo) --------------------
            O2 = []
            for c2 in range(NCC):
                psF = psum_s("ffn_ps")
                for hf in range(2):
                    for cc in range(NCC):
                        nc.tensor.matmul(
                            psF[:, hf * DV:(hf + 1) * DV],
                            WoTb[:, cc * DV + c2 * P: cc * DV + (c2 + 1) * P],
                            O1[cc][:, hf * DV:(hf + 1) * DV],
                            start=(cc == 0), stop=(cc == NCC - 1))
                trelu = pffn.tile([P, N], bf, tag="trelu", name="trelu")
                nc.scalar.activation(trelu, psF, AF.Relu,
                                     bias=bo_col[:, c2:c2 + 1])
                o2 = pffn.tile([P, N], bf, tag="o2t", name="o2t")
                nc.vector.tensor_tensor(o2, trelu, O1[c2], OP.add)
                O2.append(o2)

            # ---- LN1 + final transpose + store ---------------------------
            O3 = t_layernorm(O2, pffn, "o3t", g1_col, b1_col)
            for rc in range(NRC):
                psO = pp_s.tile([P, DV], bf, tag="s", name="out_ps")
                for cc in range(NCC):
                    nc.tensor.transpose(psO[:, cc * P:(cc + 1) * P],
                                        O3[cc][:, rc * P:(rc + 1) * P], ident_b)
                ob = pout.tile([P, DV], f32, tag="ob")
                nc.scalar.activation(ob, psO, AF.Copy)
                nc.sync.dma_start(dout[rc * P:(rc + 1) * P, :], ob)

    nc.finalize()
    return nc


def kernel(**inputs):
    from concourse.bass_utils import run_bass_kernel_spmd

    if "nc" not in _CACHE:
        _CACHE["nc"] = _build()
    nc = _CACHE["nc"]

    Q = np.ascontiguousarray(np.asarray(inputs["Q"], dtype=np.float32))
    shared = {k: np.ascontiguousarray(np.asarray(inputs[k], dtype=np.float32))
              for k in ("Wq", "bq", "Wo", "bo", "g0", "b0", "g1", "b1")}
    in_maps = [dict(Q=np.ascontiguousarray(Q[b]), **shared) for b in range(B)]

    res = run_bass_kernel_spmd(nc, in_maps, core_ids=list(range(B)),
                               **_CACHE.get("run_kwargs", {}))
    _CACHE["last_result"] = res
    return np.stack([r["out"] for r in res.results], axis=0)
